# revision 29
# baseline (speedup 1.0000x reference)
"""DANetHead Trainium2 kernel.

Sharding: 8 cores = 4 batches x 2 query-column-halves. Core c=2b+h receives
x[b] column-rolled by -2048*h, so every core runs the identical SPMD program
on "its first 2048 columns" (attention + CAM are column-permutation
equivariant). Output gathered host-side.

Per core: x f32 rounded to f32r on DVE/Pool; weights f32r via one-time
round copies; feat2/CAM branch entirely bf16. PAM in transposed-energy layout
(energyT[m,n], keys on partitions). Attention runs in 16 units/nch of 2
key-blocks each: energy into a ping-ponged 2-bank PSUM tile, exp on ScalarE
([128,1024] per unit, bf16 out, unnormalized shift-free softmax),
apply-matmuls accumulate into psO, and softmax denominators accumulate as
free-size-1 matmuls (E-chunk stationary x ones) into a [128,8] PSUM tile -
near-zero PE cost. Normalization: recip on [128,4], PE transposes to a
[1,512] row, broadcast matmul with a gamma-scaled ones stationary, then one
DVE mult + affine_then_add. k-bias dropped (softmax-invariant); q-bias
kept. vT/CAM-energy matmuls use bf16 moving operands.
"""
from contextlib import ExitStack

import numpy as np
import ml_dtypes

import concourse.bass as bass
import concourse.tile as tile
from concourse import bacc, mybir

F32 = mybir.dt.float32
F32R = mybir.dt.float32r
BF16 = mybir.dt.bfloat16
AF = mybir.ActivationFunctionType
ALU = mybir.AluOpType

B, CIN, N = 4, 512, 4096
CI, CQ, COUT = 128, 32, 256
NH = N // 2  # per-core query half

_CACHE = {}


def _build(gamma_pam: float, gamma_cam: float):
    nc = bacc.Bacc("TRN2", target_bir_lowering=False, debug=False, num_devices=8)
    dt = nc.dram_tensor
    x_d = dt("x", [CIN, N], F32, kind="ExternalInput").ap()
    waT_d = dt("waT", [128, 512], F32, kind="ExternalInput").ap()  # 4 k-tiles of Wa.T
    wcT_d = dt("wcT", [128, 512], F32, kind="ExternalInput").ap()
    wq4_d = dt("wq4", [128, 128], F32, kind="ExternalInput").ap()
    wk4_d = dt("wk4", [128, 128], F32, kind="ExternalInput").ap()
    wvT_d = dt("wvT", [128, 128], F32, kind="ExternalInput").ap()
    wa1T_d = dt("wa1T", [128, 128], F32, kind="ExternalInput").ap()
    wc1T_d = dt("wc1T", [128, 128], F32, kind="ExternalInput").ap()
    w1T_d = dt("w1T", [128, 256], F32, kind="ExternalInput").ap()
    w2T_d = dt("w2T", [128, 256], F32, kind="ExternalInput").ap()
    w3T_d = dt("w3T", [128, 512], F32, kind="ExternalInput").ap()  # 2 k-tiles side by side
    bq4_d = dt("bq4", [128, 1], F32, kind="ExternalInput").ap()
    gbv_d = dt("gbv", [128, 1], F32, kind="ExternalInput").ap()
    b3p_d = dt("b3p", [128, 2], F32, kind="ExternalInput").ap()  # col o = bias for out half o
    idenb_d = dt("idenb", [128, 128], BF16, kind="ExternalInput").ap()
    ones128_d = dt("ones128", [128, 1], BF16, kind="ExternalInput").ap()
    gones_d = dt("gones", [1, 128], F32, kind="ExternalInput").ap()  # gamma_pam row
    y_d = dt("y", [COUT, NH], F32, kind="ExternalOutput").ap()

    with tile.TileContext(nc) as tc, ExitStack() as ctx:
        wp = ctx.enter_context(tc.tile_pool(name="wp", bufs=1))
        sb = ctx.enter_context(tc.tile_pool(name="sb", bufs=1))
        stage = ctx.enter_context(tc.tile_pool(name="stage", bufs=2))
        work = ctx.enter_context(tc.tile_pool(name="work", bufs=2))

        # ---- weights: DMA fp32, one-time round to f32r on DVE ----
        def wtile(dram, shape, tag, rdtype=F32R):
            t32 = stage.tile(shape, F32, tag="wstage")
            nc.sync.dma_start(t32[:], dram)
            tr = wp.tile(shape, rdtype, tag=tag)
            nc.vector.tensor_copy(tr[:], t32[:])
            return tr

        def btile(dram, shape, tag, dtype):
            t = wp.tile(shape, dtype, tag=tag)
            nc.sync.dma_start(t[:], dram)
            return t

        waT = wtile(waT_d, [128, 512], "waT")
        wcT = wtile(wcT_d, [128, 512], "wcT")
        wq4 = wtile(wq4_d, [128, 128], "wq4")
        wk4 = wtile(wk4_d, [128, 128], "wk4")
        wvT = wtile(wvT_d, [128, 128], "wvT")
        wa1T = wtile(wa1T_d, [128, 128], "wa1T")
        wc1T = wtile(wc1T_d, [128, 128], "wc1T")
        w1T = wtile(w1T_d, [128, 256], "w1T")
        w2T = wtile(w2T_d, [128, 256], "w2T")
        w3T = wtile(w3T_d, [128, 512], "w3T")
        gones = wtile(gones_d, [1, 128], "gones")
        idenb = btile(idenb_d, [128, 128], "idenb", BF16)
        ones128 = btile(ones128_d, [128, 1], "ones128", BF16)
        bq4 = btile(bq4_d, [128, 1], "bq4", F32)
        gbv = btile(gbv_d, [128, 1], "gbv", F32)
        b3p = btile(b3p_d, [128, 2], "b3p", F32)
        idenr = wp.tile([128, 128], F32R, tag="idenr")
        nc.vector.tensor_copy(idenr[:], idenb[:])

        # ---- persistent activations ----
        feat1 = sb.tile([128, N], F32R)
        feat2 = sb.tile([128, N], BF16)

        # ================= phase A =================
        with tc.tile_pool(name="pA", bufs=4, space="PSUM") as pA:
            with tc.tile_pool(name="xp", bufs=4, space="SBUF") as xp:
                # k-outer accumulation: feat1/feat2 chunks accumulate in PSUM
                # across k so compute starts as soon as x k-tile 0 lands.
                for H in range(2):
                    xrh = []
                    ps1 = [
                        pA.tile([128, 512], F32, tag="pa", name=f"ps1_{H}_{j}")
                        for j in range(4)
                    ]
                    for k in range(4):
                        x32 = xp.tile([128, 2048], F32, tag="x32", name=f"x32_{k}")
                        nc.sync.dma_start(
                            x32[:],
                            x_d[k * 128 : (k + 1) * 128, H * 2048 : (H + 1) * 2048],
                        )
                        xrt = xp.tile([128, 2048], F32R, tag="xr", name=f"xr{k}")
                        if k == 0:
                            nc.vector.tensor_copy(xrt[:], x32[:])
                        else:
                            nc.gpsimd.tensor_copy(xrt[:], x32[:])
                        xrh.append(xrt)
                        for j in range(4):
                            nc.tensor.matmul(
                                ps1[j][:],
                                waT[:, k * 128 : (k + 1) * 128],
                                xrt[:, j * 512 : (j + 1) * 512],
                                start=(k == 0), stop=(k == 3),
                            )
                    for j in range(4):
                        nc.vector.tensor_scalar_max(
                            feat1[:, (H * 4 + j) * 512 : (H * 4 + j + 1) * 512],
                            ps1[j][:], 0.0,
                        )
                    ps2 = [
                        pA.tile([128, 512], F32, tag="pa", name=f"ps2_{H}_{j}")
                        for j in range(4)
                    ]
                    for k in range(4):
                        for j in range(4):
                            nc.tensor.matmul(
                                ps2[j][:],
                                wcT[:, k * 128 : (k + 1) * 128],
                                xrh[k][:, j * 512 : (j + 1) * 512],
                                start=(k == 0), stop=(k == 3),
                            )
                    for j in range(4):
                        nc.scalar.activation(
                            feat2[:, (H * 4 + j) * 512 : (H * 4 + j + 1) * 512],
                            ps2[j][:], AF.Relu,
                        )

            sb2 = ctx.enter_context(tc.tile_pool(name="sb2", bufs=1))
            k4 = sb2.tile([128, N], F32R)
            q4 = sb2.tile([128, NH], F32R)
            vT = sb2.tile([128, N], BF16)   # block mb at cols [mb*128,(mb+1)*128)
            f2T = sb2.tile([128, N], BF16)  # same block layout
            attnT = sb2.tile([128, 128], BF16)
            # q4 (half only, +bias) / k4 (full, bias dropped: softmax-invariant)
            for j in range(4):
                ps = pA.tile([128, 512], F32, tag="pa")
                nc.tensor.matmul(
                    ps[:], wq4[:], feat1[:, j * 512 : (j + 1) * 512],
                    start=True, stop=True,
                )
                nc.vector.tensor_scalar_add(
                    q4[:, j * 512 : (j + 1) * 512], ps[:], bq4[:]
                )
            for j in range(8):
                ps = pA.tile([128, 512], F32, tag="pa")
                nc.tensor.matmul(
                    ps[:], wk4[:], feat1[:, j * 512 : (j + 1) * 512],
                    start=True, stop=True,
                )
                nc.vector.tensor_copy(k4[:, j * 512 : (j + 1) * 512], ps[:])

            # vT blocks: vT[mb] = feat1[:,mb].T @ WvT (bf16 moving -> 1c/row)
            for g in range(8):
                ps = pA.tile([128, 512], F32, tag="pa")
                for i in range(4):
                    mb = 4 * g + i
                    nc.tensor.matmul(
                        ps[:, i * 128 : (i + 1) * 128],
                        feat1[:, mb * 128 : (mb + 1) * 128],
                        wvT[:],
                        start=True, stop=True,
                    )
                nc.vector.tensor_copy(vT[:, g * 512 : (g + 1) * 512], ps[:])

            # feat2 transposes -> f2T (all-bf16: 1c/row, 2x DVE copies)
            for g in range(8):
                ps = pA.tile([128, 512], BF16, tag="pab", bufs=2)
                for i in range(4):
                    mb = 4 * g + i
                    nc.tensor.transpose(
                        ps[:, i * 128 : (i + 1) * 128],
                        feat2[:, mb * 128 : (mb + 1) * 128],
                        idenb[:],
                    )
                nc.vector.tensor_copy(f2T[:, g * 512 : (g + 1) * 512], ps[:])

            # CAM energy + softmax + attnT
            psC = pA.tile([128, 128], F32, tag="pc", bufs=1)
            for mb in range(32):
                nc.tensor.matmul(
                    psC[:],
                    f2T[:, mb * 128 : (mb + 1) * 128],
                    f2T[:, mb * 128 : (mb + 1) * 128],
                    start=(mb == 0), stop=(mb == 31),
                )
            mn = work.tile([128, 1], F32, tag="mn")
            nc.vector.tensor_reduce(mn[:], psC[:], mybir.AxisListType.X, ALU.min)
            ex = work.tile([128, 128], F32, tag="ex")
            sm = work.tile([128, 1], F32, tag="sm")
            nc.scalar.activation(
                ex[:], psC[:], AF.Exp, bias=mn[:], scale=-1.0, accum_out=sm[:]
            )
            rec = work.tile([128, 1], F32, tag="rec")
            scr1 = work.tile([128, 1], F32, tag="scr1")
            nc.vector.reciprocal_approx_accurate(rec[:], sm[:], scr1[:])
            attn = work.tile([128, 128], BF16, tag="attn")
            nc.vector.tensor_scalar_mul(attn[:], ex[:], rec[:])
            psAT = pA.tile([128, 128], BF16, tag="pcb", bufs=1)
            nc.tensor.transpose(psAT[:], attn[:], idenb[:])
            nc.vector.tensor_copy(attnT[:], psAT[:])

        # ================= attention + CAM out =================
        with (
            tc.tile_pool(name="pE", bufs=2, space="PSUM") as pE,
            tc.tile_pool(name="pO", bufs=2, space="PSUM") as pO,
            tc.tile_pool(name="pS", bufs=1, space="PSUM") as pS,
            tc.tile_pool(name="pX", bufs=1, space="PSUM") as pX,
        ):
            # softmax denominators: col (nch%2)*4+j accumulates n-chunk j
            psSn = pS.tile([128, 8], F32)
            for nch in range(4):
                csl = slice(nch * 512, (nch + 1) * 512)
                scol = (nch % 2) * 4

                # CAM chain for this nch (independent of attention units;
                # emitted first to fill PE/DVE gaps during the unit loop)
                sc_t = work.tile([128, 512], F32R, tag="sc", name=f"sc_{nch}")
                sc2_t = work.tile([128, 512], F32R, tag="sc2", name=f"sc2_{nch}")
                psCO = pX.tile([128, 512], F32, tag="px", name=f"psCO_{nch}")
                nc.tensor.matmul(
                    psCO[:], attnT[:], feat2[:, csl], start=True, stop=True
                )
                nc.vector.scalar_tensor_tensor(
                    sc_t[:], psCO[:], float(gamma_cam), feat2[:, csl],
                    op0=ALU.mult, op1=ALU.add,
                )
                psS2 = pX.tile([128, 512], F32, tag="px", name=f"psS2_{nch}")
                nc.tensor.matmul(psS2[:], wc1T[:], sc_t[:], start=True, stop=True)
                nc.vector.tensor_scalar_max(sc2_t[:], psS2[:], 0.0)

                psO = pO.tile([128, 512], F32, tag="pO", name=f"psO_{nch}")
                nc.vector.memset(psSn[:, scol : scol + 4], 0.0)
                for u in range(16):
                    psE = pE.tile([128, 1024], F32, tag="pe", name=f"psE_{nch}_{u}")
                    for bk in range(2):
                        mb = 2 * u + bk
                        i = mb % 4
                        nc.tensor.matmul(
                            psE[:, bk * 512 : (bk + 1) * 512],
                            k4[32 * i : 32 * (i + 1), mb * 128 : (mb + 1) * 128],
                            q4[32 * i : 32 * (i + 1), csl],
                            start=True, stop=True,
                            tile_position=(32 * i, 0),
                        )
                    E = work.tile([128, 1024], BF16, tag="E", bufs=3,
                                  name=f"E_{nch}_{u}")
                    nc.scalar.activation(E[:], psE[:], AF.Exp)
                    for bk in range(2):
                        mb = 2 * u + bk
                        nc.tensor.matmul(
                            psO[:],
                            vT[:, mb * 128 : (mb + 1) * 128],
                            E[:, bk * 512 : (bk + 1) * 512],
                            start=(mb == 0), stop=(mb == 31),
                        )
                        for j in range(4):
                            nc.tensor.matmul(
                                psSn[:, scol + j : scol + j + 1],
                                E[:, bk * 512 + j * 128 : bk * 512 + (j + 1) * 128],
                                ones128[:],
                                start=False, stop=(mb == 31),
                            )

                # normalization: recip -> transpose to row -> gamma-broadcast
                rec4 = work.tile([128, 4], F32, tag="rec4", name=f"rec4_{nch}")
                scr4 = work.tile([128, 4], F32, tag="scr4", name=f"scr4_{nch}")
                nc.vector.reciprocal_approx_accurate(
                    rec4[:], psSn[:, scol : scol + 4], scr4[:]
                )
                rec4r = work.tile([128, 4], F32R, tag="rec4r", name=f"rec4r_{nch}")
                nc.vector.tensor_copy(rec4r[:], rec4[:])
                psRec = pX.tile([1, 512], F32R, tag="px", name=f"psRec_{nch}")
                for j in range(4):
                    nc.tensor.transpose(
                        psRec[0:1, j * 128 : (j + 1) * 128],
                        rec4r[:, j : j + 1],
                        idenr[:],
                    )
                srec = work.tile([1, 512], F32R, tag="srec", name=f"srec_{nch}")
                nc.vector.tensor_copy(srec[:], psRec[:])
                psB = pX.tile([128, 512], F32, tag="px", name=f"psB_{nch}")
                nc.tensor.matmul(
                    psB[:], gones[:], srec[:], start=True, stop=True
                )
                recB = work.tile([128, 512], F32, tag="recB", name=f"recB_{nch}")
                nc.vector.tensor_copy(recB[:], psB[:])
                t1 = work.tile([128, 512], F32, tag="t1", name=f"t1_{nch}")
                nc.vector.tensor_tensor(t1[:], psO[:], recB[:], op=ALU.mult)
                sa_t = work.tile([128, 512], F32R, tag="sa", name=f"sa_{nch}")
                nc.vector.affine_then_add(
                    sa_t[:], t1[:], feat1[:, csl], scale=1.0, bias=gbv[:]
                )

                # PAM tail + merge
                sa2_t = work.tile([128, 512], F32R, tag="sa2", name=f"sa2_{nch}")
                psA2 = pX.tile([128, 512], F32, tag="px", name=f"psA2_{nch}")
                nc.tensor.matmul(psA2[:], wa1T[:], sa_t[:], start=True, stop=True)
                nc.vector.tensor_scalar_max(sa2_t[:], psA2[:], 0.0)
                s_h = []
                for o in range(2):
                    psW = pX.tile([128, 512], F32, tag="px", name=f"psW_{nch}_{o}")
                    nc.tensor.matmul(psW[:], w1T[:, o * 128 : (o + 1) * 128],
                                     sa2_t[:], start=True, stop=False)
                    nc.tensor.matmul(psW[:], w2T[:, o * 128 : (o + 1) * 128],
                                     sc2_t[:], start=False, stop=True)
                    sh = work.tile([128, 512], F32R, tag="sh", name=f"sh_{nch}_{o}")
                    nc.vector.tensor_copy(sh[:], psW[:])
                    s_h.append(sh)
                for o in range(2):
                    psY = pX.tile([128, 512], F32, tag="px", name=f"psY_{nch}_{o}")
                    nc.tensor.matmul(psY[:], w3T[:, o * 128 : (o + 1) * 128],
                                     s_h[0][:], start=True, stop=False)
                    nc.tensor.matmul(psY[:], w3T[:, 256 + o * 128 : 256 + (o + 1) * 128],
                                     s_h[1][:], start=False, stop=True)
                    yt = work.tile([128, 512], F32, tag="yt", name=f"yt_{nch}_{o}")
                    nc.vector.tensor_scalar_add(yt[:], psY[:], b3p[:, o : o + 1])
                    nc.sync.dma_start(
                        y_d[o * 128 : (o + 1) * 128, csl], yt[:]
                    )

    nc.compile()
    return nc


def _build_in_maps(inputs):
    x = np.asarray(inputs["x"], dtype=np.float32)
    Wa, Wc = np.asarray(inputs["Wa"]), np.asarray(inputs["Wc"])
    Wq, bq = np.asarray(inputs["Wq"]), np.asarray(inputs["bq"])
    Wk = np.asarray(inputs["Wk"])
    Wv, bv = np.asarray(inputs["Wv"]), np.asarray(inputs["bv"])
    gp = float(np.asarray(inputs["gamma_pam"]))
    Wa1, Wc1 = np.asarray(inputs["Wa1"]), np.asarray(inputs["Wc1"])
    W1, b1 = np.asarray(inputs["W1"]), np.asarray(inputs["b1"])
    W2, b2 = np.asarray(inputs["W2"]), np.asarray(inputs["b2"])
    W3, b3 = np.asarray(inputs["W3"]), np.asarray(inputs["b3"])

    f32 = np.float32
    bf16 = ml_dtypes.bfloat16
    # k-tile k at cols [128k,128k+128): Wa.T is [512,128]; tile k = rows [128k:128k+128]
    waT = np.concatenate([Wa.T[128 * k : 128 * (k + 1), :] for k in range(4)], axis=1).astype(f32)
    wcT = np.concatenate([Wc.T[128 * k : 128 * (k + 1), :] for k in range(4)], axis=1).astype(f32)
    wq4 = np.concatenate([Wq.T] * 4, axis=1).astype(f32)  # [128, 128]
    wk4 = np.concatenate([Wk.T] * 4, axis=1).astype(f32)
    wvT = Wv.T.astype(f32)
    wa1T = Wa1.T.astype(f32)
    wc1T = Wc1.T.astype(f32)
    w1T = W1.T.astype(f32)  # [128, 256]
    w2T = W2.T.astype(f32)
    w3T = np.concatenate([W3.T[0:128, :], W3.T[128:256, :]], axis=1).astype(f32)  # [128,512]
    bq4 = np.tile(bq, 4)[:, None].astype(f32)
    gbv = (gp * bv)[:, None].astype(f32)
    b3p = (W3 @ (b1 + b2) + b3).astype(f32).reshape(2, 128).T.copy()  # [128,2]
    idenb = np.eye(128, dtype=bf16)
    ones128 = np.ones((128, 1), dtype=bf16)
    gones = np.full((1, 128), gp, dtype=f32)

    shared = dict(
        waT=waT, wcT=wcT, wq4=wq4, wk4=wk4, wvT=wvT, wa1T=wa1T, wc1T=wc1T,
        w1T=w1T, w2T=w2T, w3T=w3T, bq4=bq4, gbv=gbv, b3p=b3p,
        idenb=idenb, ones128=ones128, gones=gones,
    )
    in_maps = []
    for c in range(8):
        b, h = divmod(c, 2)
        xc = x[b] if h == 0 else np.ascontiguousarray(np.roll(x[b], -NH, axis=1))
        in_maps.append(dict(shared, x=xc.astype(f32)))
    return in_maps


def kernel(**inputs):
    gp = float(np.asarray(inputs["gamma_pam"]))
    gc = float(np.asarray(inputs["gamma_cam"]))
    key = (gp, gc)
    if key not in _CACHE:
        _CACHE[key] = _build(gp, gc)
    nc = _CACHE[key]

    in_maps = _build_in_maps(inputs)

    from concourse.bass_utils import run_bass_kernel_spmd

    res = run_bass_kernel_spmd(nc, in_maps, core_ids=list(range(8)))
    y = np.empty((B, COUT, N), dtype=np.float32)
    for c in range(8):
        b, h = divmod(c, 2)
        y[b][:, h * NH : (h + 1) * NH] = res.results[c]["y"]
    return y
